# revision 15
# baseline (speedup 1.0000x reference)
"""Low-rank causal attention on 8 TRN2 NeuronCores.

Sharding: core c -> batch b = c//4, head-group hg = c%4 (4 of 16 heads).
Per-core kernel (no collectives), software-pipelined emission: the
projection work for chunk ci+1 is emitted as small tasks interleaved
into attention chunk ci's j-loop, so every engine queue (PE, scalar,
vector, sync-DMA) sees proj(ci+1) work at priorities between att(ci)'s
j-steps and the Tile list-scheduler can gap-fill stalls.

proj(ci): qk = P(Wqk) @ x_b^T  (P = host row-permutation placing this
          core's q heads at partition stripes 32h of r-tile 0, k heads
          in r-tile 2) -> Square (scalar; same act table as Exp) ->
          transposed sum-of-squares via N=1 ones-matmuls ->
          quake-rsqrt on [128,8] (vector int bit-trick + 2 Newton
          steps; no scalar Sqrt => no 1283ns act-table reloads) ->
          qT scaled via gpsimd partition_broadcast (gpsimd's only op
          => no sw-lib thrash); v = x_b @ Wv_shard^T.
att(ci):  per (k-block j, head h): causal-width-restricted
          s = kT_h^T-slice x qT_h, pt = exp(inv_k * s) (scalar ACT,
          per-partition scale), triangular-block mask-mul (vector),
          yt[h] += v_aug^T p (ones column -> softmax denominator).
Host unshard: y_head = (yt[0:64]/max(yt[64],1e-6)).T
"""

import os
from contextlib import ExitStack

import numpy as np
import ml_dtypes

import concourse.bass as bass
from concourse import bacc
import concourse.mybir as mybir
import concourse.tile as tile
from concourse.bass_utils import run_bass_kernel_spmd

B, N, D = 2, 2048, 1024
RANK, HEADS = 256, 16
HS = RANK // HEADS          # 16
DH = D // HEADS             # 64
NCORES = 8
HPC = 4                     # heads per core
QCH = 512                   # query chunk (free dim)
KB = 128                    # key block (partition dim)
NQC = N // QCH              # 4 query chunks
NKB = N // KB               # 16 key blocks
KTILES = D // 128           # 8 contraction tiles

F32 = mybir.dt.float32
I32 = mybir.dt.int32
ALU = mybir.AluOpType

_USE_BF16 = os.environ.get("KERNEL_DT", "bf16") == "bf16"
DT = mybir.dt.bfloat16 if _USE_BF16 else mybir.dt.float32
NPDT = ml_dtypes.bfloat16 if _USE_BF16 else np.float32

_CACHE = {}
LAST_RESULT = None

RSQRT_MAGIC = 0x5F3759DF


def _build_nc():
    nc = bacc.Bacc("TRN2", target_bir_lowering=False)
    xT = nc.declare_dram_parameter("xT", [D, N], DT, isOutput=False)
    wqkT = nc.declare_dram_parameter("wqkT", [D, 2 * RANK], DT, isOutput=False)
    wvT = nc.declare_dram_parameter("wvT", [D, HPC * DH], DT, isOutput=False)
    tri = nc.declare_dram_parameter("tri", [KB, KB], DT, isOutput=False)
    out = nc.declare_dram_parameter("out", [HPC * (DH + 1), N], F32, isOutput=True)

    with tile.TileContext(nc) as tc, ExitStack() as ctx:
        const = ctx.enter_context(tc.tile_pool(name="const", bufs=1))

        xT_sb = const.tile([128, KTILES, N], DT)
        wqkT_sb = const.tile([128, KTILES, 2 * RANK], DT)
        wvT_sb = const.tile([128, KTILES, HPC * DH], DT)
        tri_sb = const.tile([128, KB], DT)

        # urgent loads on the sync queue, interleaved per contraction tile
        # so qk(0)'s kk-chain can start as soon as tile 0 lands
        for kk in range(KTILES):
            nc.sync.dma_start(
                wqkT_sb[:, kk, :], wqkT[128 * kk : 128 * kk + 128, :]
            )
            nc.sync.dma_start(
                xT_sb[:, kk, 0:QCH], xT[128 * kk : 128 * kk + 128, 0:QCH]
            )
        # bulk loads issued from the scalar queue (idle until att(0))
        nc.scalar.dma_start(tri_sb[:], tri[:, :])
        nc.scalar.dma_start(
            wvT_sb[:], wvT.rearrange("(t p) e -> p t e", p=128)
        )
        for kk in range(KTILES):
            nc.scalar.dma_start(
                xT_sb[:, kk, QCH:N], xT[128 * kk : 128 * kk + 128, QCH:N]
            )

        ones1 = const.tile([128, 1], DT)
        nc.vector.memset(ones1[:], 1.0)
        ones_i32 = const.tile([128, 8], I32)
        nc.vector.memset(ones_i32[:], 1)
        magic_i32 = const.tile([128, 8], I32)
        nc.vector.memset(magic_i32[:], RSQRT_MAGIC)

        # v with an appended ones column per head: [nk-part, ntile, head, 65]
        v_sb = const.tile([128, NKB, HPC, DH + 1], DT)
        nc.vector.memset(v_sb[:, :, :, DH : DH + 1], 1.0)

        qT_sb = const.tile([128, N], DT)   # q rows scaled by 0.25/||q||
        kT_sb = const.tile([128, N], DT)   # k rows, unnormalized
        inv_kT = const.tile([128, NKB], F32)

        pj_pool = ctx.enter_context(tc.tile_pool(name="pj", bufs=2, space="PSUM"))
        st_pool = ctx.enter_context(tc.tile_pool(name="st", bufs=2, space="PSUM"))
        yt_pool = ctx.enter_context(tc.tile_pool(name="yt", bufs=4, space="PSUM"))
        sq_pool = ctx.enter_context(tc.tile_pool(name="sq", bufs=3))
        ms_pool = ctx.enter_context(tc.tile_pool(name="ms", bufs=2))
        pt_pool = ctx.enter_context(tc.tile_pool(name="pt", bufs=2))

        def proj_tasks(ci):
            """Emission task list for chunk ci's projection + norms + v."""
            ncol = slice(QCH * ci, QCH * ci + QCH)
            tasks = []
            state = {}

            # transposed sum-of-squares: cols 0-3 q (by n-tile), 4-7 k
            def t_ss_alloc():
                state["ssT"] = st_pool.tile([128, 8], F32, tag="st", name=f"ssT{ci}")
                state["qraw"] = ms_pool.tile([128, QCH], F32, tag="qraw", name=f"qraw{ci}")
            tasks.append(t_ss_alloc)

            def t_qk(rt):
                ps = pj_pool.tile([128, QCH], F32, tag="pj", name=f"qk{ci}_{rt}")
                for kk in range(KTILES):
                    nc.tensor.matmul(
                        ps[:],
                        wqkT_sb[:, kk, 128 * rt : 128 * rt + 128],
                        xT_sb[:, kk, ncol],
                        start=(kk == 0),
                        stop=(kk == KTILES - 1),
                    )
                sq = sq_pool.tile([128, QCH], DT, tag="sq", name=f"sq{ci}_{rt}")
                if rt == 0:
                    # square from the sbuf copy on vector: keeps the scalar
                    # engine (the att pacer) free for exps
                    nc.vector.tensor_copy(state["qraw"][:], ps[:])
                    nc.vector.tensor_mul(sq[:], state["qraw"][:], state["qraw"][:])
                elif rt == 2:
                    nc.vector.tensor_copy(kT_sb[:, ncol], ps[:])
                    nc.vector.tensor_mul(sq[:], kT_sb[:, ncol], kT_sb[:, ncol])
                else:
                    nc.scalar.activation(sq[:], ps[:], mybir.ActivationFunctionType.Square)
                ssT = state["ssT"]
                half = rt // 2
                for nt in range(4):
                    nc.tensor.matmul(
                        ssT[:, 4 * half + nt : 4 * half + nt + 1],
                        sq[:, 128 * nt : 128 * nt + 128],
                        ones1[:],
                        start=(rt % 2 == 0),
                        stop=(rt % 2 == 1),
                    )
            for rt in range(4):
                tasks.append(lambda rt=rt: t_qk(rt))

            def t_rsqrt():
                ssT = state["ssT"]
                # quake rsqrt: y = bits(magic - (bits(ss) >> 1)), 2 Newton steps
                hsh = ms_pool.tile([128, 8], I32, tag="hsh")
                nc.vector.tensor_tensor(hsh[:], ssT[:].bitcast(I32), ones_i32[:], op=ALU.arith_shift_right)
                y = ms_pool.tile([128, 8], F32, tag="y")
                nc.vector.tensor_tensor(y[:].bitcast(I32), magic_i32[:], hsh[:], op=ALU.subtract)
                t = ms_pool.tile([128, 8], F32, tag="t")
                nc.vector.tensor_tensor(t[:], y[:], y[:], op=ALU.mult)
                nc.vector.tensor_tensor(t[:], t[:], ssT[:], op=ALU.mult)
                nc.vector.tensor_scalar(t[:], t[:], -0.5, 1.5, op0=ALU.mult, op1=ALU.add)
                nc.vector.tensor_tensor(y[:], y[:], t[:], op=ALU.mult)
                nc.vector.tensor_tensor(t[:], y[:], y[:], op=ALU.mult)
                nc.vector.tensor_tensor(t[:], t[:], ssT[:], op=ALU.mult)
                nc.vector.tensor_scalar(t[:], t[:], -0.5, 1.5, op0=ALU.mult, op1=ALU.add)
                # k half -> inv_kT columns; q half -> 0.25-folded qi
                nc.vector.tensor_tensor(inv_kT[:, 4 * ci : 4 * ci + 4], y[:, 4:8], t[:, 4:8], op=ALU.mult)
                qi = ms_pool.tile([128, 4], F32, tag="qi")
                nc.vector.scalar_tensor_tensor(qi[:], y[:, 0:4], 0.25, t[:, 0:4], op0=ALU.mult, op1=ALU.mult)
                qrow = ms_pool.tile([1, QCH], F32, tag="qrow")
                for bb in range(4):
                    eng = nc.sync if bb % 2 == 0 else nc.scalar
                    eng.dma_start(
                        qrow[0:1, 128 * bb : 128 * bb + 128], qi[:, bb : bb + 1]
                    )
                qbc = ms_pool.tile([128, QCH], F32, tag="qbc")
                nc.gpsimd.partition_broadcast(qbc[:], qrow[0:1, :])
                nc.vector.tensor_mul(qT_sb[:, ncol], state["qraw"][:], qbc[:])
            tasks.append(t_rsqrt)

            def t_v(nt):
                vp = pj_pool.tile([128, HPC * DH], F32, tag="pj", name=f"v{nt}")
                for kk in range(KTILES):
                    nc.tensor.matmul(
                        vp[:],
                        xT_sb[:, kk, 128 * nt : 128 * nt + 128],
                        wvT_sb[:, kk, :],
                        start=(kk == 0),
                        stop=(kk == KTILES - 1),
                    )
                nc.vector.tensor_copy(
                    v_sb[:, nt, :, 0:DH],
                    vp[:].rearrange("p (h e) -> p h e", h=HPC),
                )
            for nt in range(NQC * ci, NQC * ci + NQC):
                tasks.append(lambda nt=nt: t_v(nt))
            return tasks

        def emit_att(ci, fillers):
            ncol = slice(QCH * ci, QCH * ci + QCH)
            nj = NQC * ci + NQC  # causal: k-blocks 0 .. 4*ci+3
            yts = [
                yt_pool.tile([DH + 1, QCH], F32, name=f"yt{ci}_{h}", tag="yt")
                for h in range(HPC)
            ]
            per_j = (len(fillers) + nj - 1) // nj if fillers else 0
            fi = 0
            for j in range(nj):
                d = j - NQC * ci  # >= 0: diagonal band block
                off = 128 * d if d > 0 else 0
                w = QCH - off
                pts = []
                for h in range(HPC):
                    stt = st_pool.tile([128, QCH], F32, tag="st", name=f"st{j}_{h}")
                    nc.tensor.matmul(
                        stt[:, off:QCH],
                        kT_sb[32 * h : 32 * h + HS, 128 * j : 128 * j + 128],
                        qT_sb[32 * h : 32 * h + HS, QCH * ci + off : QCH * ci + QCH],
                        start=True,
                        stop=True,
                        tile_position=(32 * h, 0),
                    )
                    pt = pt_pool.tile(
                        [128, w], DT, tag=f"p{max(d, 0)}",
                        bufs=(6 if d <= 0 else 2), name=f"pt{j}_{h}",
                    )
                    nc.scalar.activation(
                        pt[:],
                        stt[:, off:QCH],
                        mybir.ActivationFunctionType.Exp,
                        scale=inv_kT[:, j : j + 1],
                    )
                    if d >= 0:
                        nc.vector.tensor_mul(pt[:, 0:KB], pt[:, 0:KB], tri_sb[:])
                    pts.append(pt)
                # pv matmuls with proj(ci+1) filler tasks between them: the
                # fillers keep the PE busy while the exps/tri-muls land
                budget = min(per_j, len(fillers) - fi)
                for h in range(HPC):
                    if h > 0 and budget > 0:
                        fillers[fi]()
                        fi += 1
                        budget -= 1
                    nc.tensor.matmul(
                        yts[h][:, off:QCH],
                        v_sb[:, j, h, :],
                        pts[h][:],
                        start=(j == 0),
                        stop=(j == nj - 1),
                        skip_group_check=True,
                    )
                for _ in range(budget):
                    fillers[fi]()
                    fi += 1
            while fi < len(fillers):
                fillers[fi]()
                fi += 1
            for h in range(HPC):
                yo = ms_pool.tile([DH + 1, QCH], F32, name=f"yo{ci}_{h}", tag="yo", bufs=4)
                nc.vector.tensor_copy(yo[:], yts[h][:])
                nc.sync.dma_start(
                    out[(DH + 1) * h : (DH + 1) * (h + 1), ncol], yo[:]
                )

        for t in proj_tasks(0):
            t()
        for ci in range(NQC):
            fillers = proj_tasks(ci + 1) if ci + 1 < NQC else []
            emit_att(ci, fillers)
    nc.compile()
    return nc


def _perm_for_core(hg: int) -> np.ndarray:
    """Row permutation of Wqk: this core's q heads land at partition stripes
    32h (h=0..3) of output r-tile 0, its k heads likewise in r-tile 2."""
    perm = np.empty(2 * RANK, dtype=np.int64)
    for part, base in ((0, 0), (1, RANK)):  # q rows then k rows
        pos_used = np.zeros(RANK, dtype=bool)
        for h in range(HPC):
            head = HPC * hg + h
            rows = base + HS * head + np.arange(HS)
            perm[base + 32 * h : base + 32 * h + HS] = rows
            pos_used[32 * h : 32 * h + HS] = True
        fill_rows = [
            base + HS * head + r
            for head in range(HEADS)
            if head not in range(HPC * hg, HPC * hg + HPC)
            for r in range(HS)
        ]
        fill_pos = np.flatnonzero(~pos_used)
        perm[base + fill_pos] = fill_rows
    return perm


def kernel(x, mask, Wqk, Wv):
    global LAST_RESULT
    x = np.asarray(x)
    Wqk = np.asarray(Wqk)
    Wv = np.asarray(Wv)

    if "nc" not in _CACHE:
        _CACHE["nc"] = _build_nc()
    nc = _CACHE["nc"]

    tri01 = np.triu(np.ones((KB, KB), dtype=np.float32)).astype(NPDT)

    in_maps = []
    for c in range(NCORES):
        b, hg = divmod(c, HPC)
        perm = _perm_for_core(hg)
        in_maps.append(
            {
                "xT": np.ascontiguousarray(x[b].T).astype(NPDT),
                "wqkT": np.ascontiguousarray(Wqk[perm].T).astype(NPDT),
                "wvT": np.ascontiguousarray(
                    Wv[DH * HPC * hg : DH * HPC * (hg + 1)].T
                ).astype(NPDT),
                "tri": tri01,
            }
        )

    trace = bool(os.environ.get("KBENCH_TRACE"))
    res = run_bass_kernel_spmd(nc, in_maps, list(range(NCORES)), trace=trace)
    LAST_RESULT = res

    y = np.empty((B, N, D), dtype=np.float32)
    for c in range(NCORES):
        b, hg = divmod(c, HPC)
        arr = res.results[c]["out"]
        for h in range(HPC):
            num = arr[(DH + 1) * h : (DH + 1) * h + DH]          # [64, N]
            den = np.maximum(arr[(DH + 1) * h + DH], 1e-6)       # [N]
            head = HPC * hg + h
            y[b, :, DH * head : DH * (head + 1)] = (num / den).T
    return y
